# revision 12
# baseline (speedup 1.0000x reference)
"""Causal multi-head attention block (B=2, T=2048, C=1024, H=16) on 8 TRN2
NeuronCores.

Sharding: Megatron-style tensor parallel over heads. Core r owns heads
{2r, 2r+1} (output dims [128r, 128r+128) of Wq/Wk/Wv). The final output
projection contracts over all of C, so cores AllGather their local yT
shards (concat on the partition axis == feature axis) into yT_full
[C, B*T], then each core computes a 128-column shard of the output:
outT_shard = Wo[128r:128r+128, :] @ yT_full.

Everything on-device is computed in the "transposed" orientation
(feature-major, token-minor) so the TensorEngine contraction axis always
sits on SBUF partitions and the softmax denominator arrives for free via
a ones-column appended to V:

  qT/kT/vT [128, 4096] = W_shard @ x^T          (x^T passed from host)
  ST tile [128k, 512q] = kT_slice.T @ qT_slice  (contract d=64)
  PT = exp(ST * 1/sqrt(d))                      (no max-subtraction: logits
                                                 are ~N(0,1), |S|max ~ 6)
  causal mask: zero PT where k > q via gpsimd.affine_select
  yT [65, 512] += [v | 1].T @ PT                (row 64 = softmax denom)
  yT_norm = yT[0:64] / broadcast(yT[64])

k-tiles are processed in pairs sharing one 2-bank PSUM tile so each EXP
covers 1024 columns (the ACT engine has a ~352-cycle fixed cost per
instruction). The AllGather is split into 4 chunks (per batch x half) so
gather and output projection overlap the remaining attention compute.

Inputs are bf16 (host-side cast); accumulation is f32 in PSUM.
"""

import numpy as np
import ml_dtypes

import concourse.bacc as bacc
import concourse.mybir as mybir
import concourse.tile as tile
from concourse.bass_utils import run_bass_kernel_spmd
from concourse.masks import make_identity

N_CORES = 8
B, T, C, H = 2, 2048, 1024, 16
D = 64                # head dim
HL = H // N_CORES     # heads per core = 2
DL = HL * D           # local feature dim = 128
TT = B * T            # 4096 tokens total
P = 128
NCH = C // P          # 8 contraction chunks
QCH = 512             # q-chunk (moving free dim)
NQC = T // QCH        # 4 q-chunks per batch entry
NKT = T // P          # 16 k-tiles per batch entry
HCH = T // 2          # AllGather chunk = half batch-entry = 1024 tokens
SCALE = 1.0 / np.sqrt(D)

BF = mybir.dt.bfloat16
F32 = mybir.dt.float32
AF = mybir.ActivationFunctionType


def build_graph():
    nc = bacc.Bacc("TRN2", target_bir_lowering=False, debug=False)

    xT = nc.dram_tensor("xT", [C, TT], BF, kind="ExternalInput")
    # all 4 weight shards pre-packed host-side into SBUF layout
    # [p, w, ci, m]: one contiguous 1MB DMA instead of 4x1024 tiny rows
    wall = nc.dram_tensor("wall", [P, 4 * NCH * DL], BF, kind="ExternalInput")
    out = nc.dram_tensor("out", [DL, TT], F32, kind="ExternalOutput")

    with tile.TileContext(nc) as tc:
        with (
            tc.tile_pool(name="sb", bufs=1) as sb,
            tc.tile_pool(name="ps", bufs=1, space="PSUM") as ps,
            tc.tile_pool(name="dram", bufs=1, space="DRAM") as dram,
        ):
            # ---- phase 0: loads ----
            w_sb = sb.tile([P, 4, NCH, DL], BF, name="w_sb")
            nc.sync.dma_start(
                w_sb[:], wall[:].rearrange("p (w a m) -> p w a m", w=4, a=NCH)
            )
            wq_sb, wk_sb, wv_sb, wo_sb = (w_sb[:, i] for i in range(4))

            ident = sb.tile([P, P], BF, name="ident")
            make_identity(nc, ident)
            # strictly-lower-triangular -1e9 (k > q) used to mask causal
            # logits on the diagonal blocks, injected into the St PSUM
            # accumulation group via matmul(ident, mneg)
            mneg = sb.tile([P, P], BF, name="mneg")
            nc.gpsimd.memset(mneg[:], 0.0)
            nc.gpsimd.affine_select(
                out=mneg[:],
                in_=mneg[:],
                compare_op=mybir.AluOpType.is_ge,
                fill=-1e9,
                base=0,
                channel_multiplier=-1,
                pattern=[[1, P]],
            )
            # [1, 64] f32 ones: K=1 outer-product broadcast of the softmax
            # denominator across partitions on the TensorEngine
            ones64 = sb.tile([1, D], F32, name="ones64")
            nc.vector.memset(ones64[:], 1.0)

            qT_sb = sb.tile([P, TT], BF, name="qT_sb")
            kT_sb = sb.tile([P, TT], BF, name="kT_sb")
            vT_sb = sb.tile([P, TT], BF, name="vT_sb")
            # v in natural layout, packed per 128-token tile as
            # [headA(64) | 1 | headB(64) | 1] -> 130 columns
            v_sb = sb.tile([P, TT // P, 2 * (D + 1)], BF, name="v_sb")
            nc.gpsimd.memset(v_sb[:], 1.0)

            yT_all = sb.tile([P, TT], BF, name="yT_all")

            with tc.tile_pool(name="xp", bufs=1) as xp:
                xT_sb = xp.tile([P, NCH, TT], BF, name="xT_sb")
                # half-major so the first projections' operands land first,
                # with 4KB descriptors (2048-col rows)
                for hf in range(2):
                    hsl = slice(hf * (TT // 2), (hf + 1) * (TT // 2))
                    for ci in range(NCH):
                        nc.sync.dma_start(
                            xT_sb[:, ci, hsl], xT[ci * P:(ci + 1) * P, hsl]
                        )

                # ---- phase 1: QKV projections (transposed layout) ----
                for tch in range(TT // QCH):
                    tsl = slice(tch * QCH, (tch + 1) * QCH)
                    for wsb, dst in (
                        (wq_sb, qT_sb), (wk_sb, kT_sb), (wv_sb, vT_sb)
                    ):
                        pj = ps.tile([P, QCH], F32, tag="mm", bufs=2)
                        for ci in range(NCH):
                            nc.tensor.matmul(
                                pj[:],
                                wsb[:, ci, :],
                                xT_sb[:, ci, tsl],
                                start=(ci == 0),
                                stop=(ci == NCH - 1),
                            )
                        nc.vector.tensor_copy(dst[:, tsl], pj[:])
                    # transpose this chunk of vT into v_sb (natural layout)
                    for t32 in range(tch * (QCH // P), (tch + 1) * (QCH // P)):
                        tr = ps.tile([P, P], BF, tag="mm", bufs=2)
                        nc.tensor.transpose(
                            tr[:], vT_sb[:, t32 * P:(t32 + 1) * P], ident[:]
                        )
                        out_ap = v_sb[:, t32, :].rearrange(
                            "p (h x) -> p h x", h=HL
                        )[:, :, 0:D]
                        in_ap = tr[:].rearrange("p (h x) -> p h x", h=HL)
                        nc.vector.tensor_copy(out_ap, in_ap)

                # ---- phase 2+3+4: attention, chunked AllGather, O-proj ----
                ag_in = [
                    dram.tile([DL, QCH], BF, name=f"ag_in{c}")
                    for c in range(8)
                ]
                ytf = [
                    dram.tile([C, QCH], BF, name=f"ytf{c}", addr_space="Shared")
                    for c in range(8)
                ]

                def attn_chunk(b, jq, h):
                    rsl = slice(h * D, (h + 1) * D)
                    q0 = b * T + jq * QCH
                    yt = ps.tile([D + 1, QCH], F32, tag="yt", bufs=2)
                    nkt = 4 * jq + 4
                    for pr in range(nkt // 2):
                        st = ps.tile([P, 2 * QCH], F32, tag="st", bufs=2)
                        pt = sb.tile([P, 2 * QCH], BF, tag="pt", bufs=3)
                        for half in range(2):
                            kt = 2 * pr + half
                            k0 = b * T + kt * P
                            i = kt - 4 * jq
                            # diagonal tiles: only q >= kt*128 is live; the
                            # leading 128 live columns get the -1e9 triangle
                            qv = max(i, 0) * P
                            ssl = slice(half * QCH + qv, (half + 1) * QCH)
                            nc.tensor.matmul(
                                st[:, ssl],
                                kT_sb[rsl, k0:k0 + P],
                                qT_sb[rsl, q0 + qv:q0 + QCH],
                                start=True,
                                stop=(i < 0),
                            )
                            if i >= 0:
                                nc.tensor.matmul(
                                    st[:, half * QCH + qv:
                                       half * QCH + qv + P],
                                    ident[:],
                                    mneg[:],
                                    start=False,
                                    stop=True,
                                )
                        nc.scalar.activation(
                            pt[:], st[:], AF.Exp, scale=float(SCALE)
                        )
                        for half in range(2):
                            kt = 2 * pr + half
                            qv = max(kt - 4 * jq, 0) * P
                            nc.tensor.matmul(
                                yt[:, qv:QCH],
                                v_sb[:, b * NKT + kt,
                                     h * (D + 1):(h + 1) * (D + 1)],
                                pt[:, half * QCH + qv:(half + 1) * QCH],
                                start=(kt == 0),
                                stop=(kt == nkt - 1),
                            )
                    # normalize: yT[0:64] * recip(yT[64]) (denominator row).
                    # Broadcast across partitions via a K=1 f32 outer product
                    # on the TensorEngine (gpsimd is reserved for collectives).
                    den = sb.tile([1, QCH], F32, tag="den", bufs=3)
                    nc.vector.tensor_copy(den[:], yt[D:D + 1, :])
                    bc = ps.tile([D, QCH], F32, tag="mm", bufs=2)
                    nc.tensor.matmul(
                        bc[:], ones64[:], den[:], start=True, stop=True
                    )
                    rcp = sb.tile([D, QCH], F32, tag="rcp", bufs=3)
                    scr = sb.tile([D, QCH], F32, tag="scr", bufs=3)
                    nc.vector.reciprocal_approx_accurate(
                        rcp[:], bc[:], scratch=scr[:]
                    )
                    nc.vector.tensor_mul(
                        yT_all[rsl, q0:q0 + QCH], yt[0:D, :], rcp[:]
                    )

                def oproj_chunk(c):
                    # gather chunk c (tokens [c*QCH, (c+1)*QCH)) and compute
                    # the local 128-column shard of the output projection.
                    # Collective issued from the sync queue so it doesn't
                    # block gpsimd's affine_select/broadcast stream.
                    csl = slice(c * QCH, (c + 1) * QCH)
                    nc.sync.dma_start(ag_in[c][:], yT_all[:, csl])
                    nc.gpsimd.collective_compute(
                        "AllGather",
                        mybir.AluOpType.bypass,
                        replica_groups=[list(range(N_CORES))],
                        ins=[ag_in[c][:]],
                        outs=[ytf[c][:]],
                    )
                    yf = sb.tile([P, NCH, QCH], BF, tag="yf", bufs=2)
                    for ci in range(NCH):
                        nc.sync.dma_start(
                            yf[:, ci, :], ytf[c][ci * P:(ci + 1) * P, :]
                        )
                    po = ps.tile([P, QCH], F32, tag="mm", bufs=2)
                    for ci in range(NCH):
                        nc.tensor.matmul(
                            po[:],
                            wo_sb[:, ci, :],
                            yf[:, ci, :],
                            start=(ci == 0),
                            stop=(ci == NCH - 1),
                        )
                    ob = sb.tile([P, QCH], F32, tag="ob", bufs=2)
                    nc.vector.tensor_copy(ob[:], po[:])
                    nc.sync.dma_start(out[:, csl], ob[:])

                for b in range(B):
                    for jq in range(NQC):
                        for h in range(HL):
                            attn_chunk(b, jq, h)
                        oproj_chunk(b * NQC + jq)

    nc.finalize()
    return nc


_GRAPH = None


def _get_graph():
    global _GRAPH
    if _GRAPH is None:
        _GRAPH = build_graph()
    return _GRAPH


def prepare_in_maps(x, Wq, Wk, Wv, Wo):
    x = np.asarray(x, np.float32)
    Wq = np.asarray(Wq, np.float32)
    Wk = np.asarray(Wk, np.float32)
    Wv = np.asarray(Wv, np.float32)
    Wo = np.asarray(Wo, np.float32)

    bf = ml_dtypes.bfloat16
    xTh = np.ascontiguousarray(x.reshape(TT, C).T).astype(bf)
    in_maps = []
    for r in range(N_CORES):
        sl = slice(r * DL, (r + 1) * DL)
        # pack the 4 transposed weight shards into the SBUF layout
        # [p, w, ci, m] where the shard row index is c = ci*128 + p
        wall = np.empty((P, 4, NCH, DL), np.float32)
        for w, W in enumerate((Wq, Wk, Wv, Wo)):
            wall[:, w] = W[sl].T.reshape(NCH, P, DL).transpose(1, 0, 2)
        in_maps.append({
            "xT": xTh,
            "wall": np.ascontiguousarray(
                wall.reshape(P, 4 * NCH * DL)).astype(bf),
        })
    return in_maps


def assemble_output(results):
    outT = np.concatenate(
        [np.asarray(results[r]["out"], np.float32) for r in range(N_CORES)],
        axis=0,
    )  # [C, TT]
    return np.ascontiguousarray(outT.T).reshape(B, T, C)


def kernel(x, Wq, Wk, Wv, Wo):
    nc = _get_graph()
    in_maps = prepare_in_maps(x, Wq, Wk, Wv, Wo)
    res = run_bass_kernel_spmd(nc, in_maps, core_ids=list(range(N_CORES)))
    return assemble_output(res.results)


# revision 13
# speedup vs baseline: 1.0088x; 1.0088x over previous
"""Causal multi-head attention block (B=2, T=2048, C=1024, H=16) on 8 TRN2
NeuronCores.

Sharding: Megatron-style tensor parallel over heads. Core r owns heads
{2r, 2r+1} (output dims [128r, 128r+128) of Wq/Wk/Wv). The final output
projection contracts over all of C, so cores AllGather their local yT
shards (concat on the partition axis == feature axis) into yT_full
[C, B*T], then each core computes a 128-column shard of the output:
outT_shard = Wo[128r:128r+128, :] @ yT_full.

Everything on-device is computed in the "transposed" orientation
(feature-major, token-minor) so the TensorEngine contraction axis always
sits on SBUF partitions and the softmax denominator arrives for free via
a ones-column appended to V:

  qT/kT/vT [128, 4096] = W_shard @ x^T          (x^T passed from host)
  ST tile [128k, 512q] = kT_slice.T @ qT_slice  (contract d=64)
  PT = exp(ST * 1/sqrt(d))                      (no max-subtraction: logits
                                                 are ~N(0,1), |S|max ~ 6)
  causal mask: zero PT where k > q via gpsimd.affine_select
  yT [65, 512] += [v | 1].T @ PT                (row 64 = softmax denom)
  yT_norm = yT[0:64] / broadcast(yT[64])

k-tiles are processed in pairs sharing one 2-bank PSUM tile so each EXP
covers 1024 columns (the ACT engine has a ~352-cycle fixed cost per
instruction). The AllGather is split into 4 chunks (per batch x half) so
gather and output projection overlap the remaining attention compute.

Inputs are bf16 (host-side cast); accumulation is f32 in PSUM.
"""

import numpy as np
import ml_dtypes

import concourse.bacc as bacc
import concourse.mybir as mybir
import concourse.tile as tile
from concourse.bass_utils import run_bass_kernel_spmd
from concourse.masks import make_identity

N_CORES = 8
B, T, C, H = 2, 2048, 1024, 16
D = 64                # head dim
HL = H // N_CORES     # heads per core = 2
DL = HL * D           # local feature dim = 128
TT = B * T            # 4096 tokens total
P = 128
NCH = C // P          # 8 contraction chunks
QCH = 512             # q-chunk (moving free dim)
NQC = T // QCH        # 4 q-chunks per batch entry
NKT = T // P          # 16 k-tiles per batch entry
HCH = T // 2          # AllGather chunk = half batch-entry = 1024 tokens
SCALE = 1.0 / np.sqrt(D)

BF = mybir.dt.bfloat16
F32 = mybir.dt.float32
AF = mybir.ActivationFunctionType


def build_graph():
    nc = bacc.Bacc("TRN2", target_bir_lowering=False, debug=False)

    xT = nc.dram_tensor("xT", [C, TT], BF, kind="ExternalInput")
    # all 4 weight shards pre-packed host-side into SBUF layout
    # [p, w, ci, m]: one contiguous 1MB DMA instead of 4x1024 tiny rows
    wall = nc.dram_tensor("wall", [P, 4 * NCH * DL], BF, kind="ExternalInput")
    out = nc.dram_tensor("out", [DL, TT], F32, kind="ExternalOutput")

    with tile.TileContext(nc) as tc:
        with (
            tc.tile_pool(name="sb", bufs=1) as sb,
            tc.tile_pool(name="ps", bufs=1, space="PSUM") as ps,
            tc.tile_pool(name="dram", bufs=1, space="DRAM") as dram,
        ):
            # ---- phase 0: loads ----
            w_sb = sb.tile([P, 4, NCH, DL], BF, name="w_sb")
            nc.sync.dma_start(
                w_sb[:], wall[:].rearrange("p (w a m) -> p w a m", w=4, a=NCH)
            )
            wq_sb, wk_sb, wv_sb, wo_sb = (w_sb[:, i] for i in range(4))

            ident = sb.tile([P, P], BF, name="ident")
            make_identity(nc, ident)
            # strictly-lower-triangular -1e9 (k > q) used to mask causal
            # logits on the diagonal blocks, injected into the St PSUM
            # accumulation group via matmul(ident, mneg)
            mneg = sb.tile([P, P], BF, name="mneg")
            nc.gpsimd.memset(mneg[:], 0.0)
            nc.gpsimd.affine_select(
                out=mneg[:],
                in_=mneg[:],
                compare_op=mybir.AluOpType.is_ge,
                fill=-1e9,
                base=0,
                channel_multiplier=-1,
                pattern=[[1, P]],
            )
            # [1, 64] f32 ones: K=1 outer-product broadcast of the softmax
            # denominator across partitions on the TensorEngine
            ones64 = sb.tile([1, D], F32, name="ones64")
            nc.vector.memset(ones64[:], 1.0)

            qT_sb = sb.tile([P, TT], BF, name="qT_sb")
            kT_sb = sb.tile([P, TT], BF, name="kT_sb")
            vT_sb = sb.tile([P, TT], BF, name="vT_sb")
            # v in natural layout, packed per 128-token tile as
            # [headA(64) | 1 | headB(64) | 1] -> 130 columns
            v_sb = sb.tile([P, TT // P, 2 * (D + 1)], BF, name="v_sb")
            nc.gpsimd.memset(v_sb[:], 1.0)

            yT_all = sb.tile([P, TT], BF, name="yT_all")

            with tc.tile_pool(name="xp", bufs=1) as xp:
                xT_sb = xp.tile([P, NCH, TT], BF, name="xT_sb")
                # half-major so the first projections' operands land first,
                # with 4KB descriptors (2048-col rows)
                for hf in range(2):
                    hsl = slice(hf * (TT // 2), (hf + 1) * (TT // 2))
                    for ci in range(NCH):
                        nc.sync.dma_start(
                            xT_sb[:, ci, hsl], xT[ci * P:(ci + 1) * P, hsl]
                        )

                # ---- phase 1: QKV projections (transposed layout) ----
                for tch in range(TT // QCH):
                    tsl = slice(tch * QCH, (tch + 1) * QCH)
                    for wsb, dst in (
                        (wq_sb, qT_sb), (wk_sb, kT_sb), (wv_sb, vT_sb)
                    ):
                        pj = ps.tile([P, QCH], F32, tag="st", bufs=2)
                        for ci in range(NCH):
                            nc.tensor.matmul(
                                pj[:],
                                wsb[:, ci, :],
                                xT_sb[:, ci, tsl],
                                start=(ci == 0),
                                stop=(ci == NCH - 1),
                            )
                        nc.vector.tensor_copy(dst[:, tsl], pj[:])
                    # transpose this chunk of vT into v_sb (natural layout)
                    for t32 in range(tch * (QCH // P), (tch + 1) * (QCH // P)):
                        tr = ps.tile([P, P], BF, tag="st", bufs=2)
                        nc.tensor.transpose(
                            tr[:], vT_sb[:, t32 * P:(t32 + 1) * P], ident[:]
                        )
                        out_ap = v_sb[:, t32, :].rearrange(
                            "p (h x) -> p h x", h=HL
                        )[:, :, 0:D]
                        in_ap = tr[:].rearrange("p (h x) -> p h x", h=HL)
                        nc.vector.tensor_copy(out_ap, in_ap)

                # ---- phase 2+3+4: attention, chunked AllGather, O-proj ----
                ag_in = [
                    dram.tile([DL, HCH], BF, name=f"ag_in{c}")
                    for c in range(4)
                ]
                ytf = [
                    dram.tile([C, HCH], BF, name=f"ytf{c}", addr_space="Shared")
                    for c in range(4)
                ]

                def attn_chunk(b, jq, h):
                    rsl = slice(h * D, (h + 1) * D)
                    q0 = b * T + jq * QCH
                    yt = ps.tile([D + 1, QCH], F32, tag="yt", bufs=2)
                    nkt = 4 * jq + 4
                    for pr in range(nkt // 2):
                        st = ps.tile([P, 2 * QCH], F32, tag="st", bufs=2)
                        pt = sb.tile([P, 2 * QCH], BF, tag="pt", bufs=3)
                        for half in range(2):
                            kt = 2 * pr + half
                            k0 = b * T + kt * P
                            i = kt - 4 * jq
                            # diagonal tiles: only q >= kt*128 is live; the
                            # leading 128 live columns get the -1e9 triangle
                            qv = max(i, 0) * P
                            ssl = slice(half * QCH + qv, (half + 1) * QCH)
                            nc.tensor.matmul(
                                st[:, ssl],
                                kT_sb[rsl, k0:k0 + P],
                                qT_sb[rsl, q0 + qv:q0 + QCH],
                                start=True,
                                stop=(i < 0),
                            )
                            if i >= 0:
                                nc.tensor.matmul(
                                    st[:, half * QCH + qv:
                                       half * QCH + qv + P],
                                    ident[:],
                                    mneg[:],
                                    start=False,
                                    stop=True,
                                )
                        nc.scalar.activation(
                            pt[:], st[:], AF.Exp, scale=float(SCALE)
                        )
                        for half in range(2):
                            kt = 2 * pr + half
                            qv = max(kt - 4 * jq, 0) * P
                            nc.tensor.matmul(
                                yt[:, qv:QCH],
                                v_sb[:, b * NKT + kt,
                                     h * (D + 1):(h + 1) * (D + 1)],
                                pt[:, half * QCH + qv:(half + 1) * QCH],
                                start=(kt == 0),
                                stop=(kt == nkt - 1),
                            )
                    # normalize: yT[0:64] * recip(yT[64]) (denominator row).
                    # Broadcast across partitions via a K=1 f32 outer product
                    # on the TensorEngine (gpsimd is reserved for collectives).
                    den = sb.tile([1, QCH], F32, tag="den", bufs=3)
                    nc.vector.tensor_copy(den[:], yt[D:D + 1, :])
                    bc = ps.tile([D, QCH], F32, tag="st", bufs=2)
                    nc.tensor.matmul(
                        bc[:], ones64[:], den[:], start=True, stop=True
                    )
                    rcp = sb.tile([D, QCH], F32, tag="rcp", bufs=3)
                    scr = sb.tile([D, QCH], F32, tag="scr", bufs=3)
                    nc.vector.reciprocal_approx_accurate(
                        rcp[:], bc[:], scratch=scr[:]
                    )
                    nc.vector.tensor_mul(
                        yT_all[rsl, q0:q0 + QCH], yt[0:D, :], rcp[:]
                    )

                def oproj_chunk(c):
                    # gather chunk c (tokens [c*QCH, (c+1)*QCH)) and compute
                    # the local 128-column shard of the output projection.
                    # Collective issued from the sync queue so it doesn't
                    # block gpsimd's affine_select/broadcast stream.
                    csl = slice(c * HCH, (c + 1) * HCH)
                    nc.sync.dma_start(ag_in[c][:], yT_all[:, csl])
                    nc.gpsimd.collective_compute(
                        "AllGather",
                        mybir.AluOpType.bypass,
                        replica_groups=[list(range(N_CORES))],
                        ins=[ag_in[c][:]],
                        outs=[ytf[c][:]],
                    )
                    yf = sb.tile([P, NCH, HCH], BF, tag="yf", bufs=2)
                    for ci in range(NCH):
                        nc.sync.dma_start(
                            yf[:, ci, :], ytf[c][ci * P:(ci + 1) * P, :]
                        )
                    for tch in range(HCH // QCH):
                        po = ps.tile([P, QCH], F32, tag="po", bufs=2)
                        for ci in range(NCH):
                            nc.tensor.matmul(
                                po[:],
                                wo_sb[:, ci, :],
                                yf[:, ci, tch * QCH:(tch + 1) * QCH],
                                start=(ci == 0),
                                stop=(ci == NCH - 1),
                            )
                        ob = sb.tile([P, QCH], F32, tag="ob", bufs=2)
                        nc.vector.tensor_copy(ob[:], po[:])
                        nc.sync.dma_start(
                            out[:, c * HCH + tch * QCH:
                                c * HCH + (tch + 1) * QCH],
                            ob[:],
                        )

                for b in range(B):
                    for jq in range(NQC):
                        for h in range(HL):
                            attn_chunk(b, jq, h)
                        if jq % 2 == 1:
                            oproj_chunk(b * 2 + jq // 2)

    nc.finalize()
    return nc


_GRAPH = None


def _get_graph():
    global _GRAPH
    if _GRAPH is None:
        _GRAPH = build_graph()
    return _GRAPH


def prepare_in_maps(x, Wq, Wk, Wv, Wo):
    x = np.asarray(x, np.float32)
    Wq = np.asarray(Wq, np.float32)
    Wk = np.asarray(Wk, np.float32)
    Wv = np.asarray(Wv, np.float32)
    Wo = np.asarray(Wo, np.float32)

    bf = ml_dtypes.bfloat16
    xTh = np.ascontiguousarray(x.reshape(TT, C).T).astype(bf)
    in_maps = []
    for r in range(N_CORES):
        sl = slice(r * DL, (r + 1) * DL)
        # pack the 4 transposed weight shards into the SBUF layout
        # [p, w, ci, m] where the shard row index is c = ci*128 + p
        wall = np.empty((P, 4, NCH, DL), np.float32)
        for w, W in enumerate((Wq, Wk, Wv, Wo)):
            wall[:, w] = W[sl].T.reshape(NCH, P, DL).transpose(1, 0, 2)
        in_maps.append({
            "xT": xTh,
            "wall": np.ascontiguousarray(
                wall.reshape(P, 4 * NCH * DL)).astype(bf),
        })
    return in_maps


def assemble_output(results):
    outT = np.concatenate(
        [np.asarray(results[r]["out"], np.float32) for r in range(N_CORES)],
        axis=0,
    )  # [C, TT]
    return np.ascontiguousarray(outT.T).reshape(B, T, C)


def kernel(x, Wq, Wk, Wv, Wo):
    nc = _get_graph()
    in_maps = prepare_in_maps(x, Wq, Wk, Wv, Wo)
    res = run_bass_kernel_spmd(nc, in_maps, core_ids=list(range(N_CORES)))
    return assemble_output(res.results)


# revision 14
# speedup vs baseline: 1.2166x; 1.2059x over previous
"""Causal multi-head attention block (B=2, T=2048, C=1024, H=16) on 8 TRN2
NeuronCores.

Sharding: Megatron-style tensor parallel over heads. Core r owns heads
{2r, 2r+1} (output dims [128r, 128r+128) of Wq/Wk/Wv). The final output
projection contracts over all of C, so cores AllGather their local yT
shards (concat on the partition axis == feature axis) into yT_full
[C, B*T], then each core computes a 128-column shard of the output:
outT_shard = Wo[128r:128r+128, :] @ yT_full.

Everything on-device is computed in the "transposed" orientation
(feature-major, token-minor) so the TensorEngine contraction axis always
sits on SBUF partitions and the softmax denominator arrives for free via
a ones-column appended to V:

  qT/kT/vT [128, 4096] = W_shard @ x^T          (x^T passed from host)
  ST tile [128k, 512q] = kT_slice.T @ qT_slice  (contract d=64)
  PT = exp(ST * 1/sqrt(d))                      (no max-subtraction: logits
                                                 are ~N(0,1), |S|max ~ 6)
  causal mask: zero PT where k > q via gpsimd.affine_select
  yT [65, 512] += [v | 1].T @ PT                (row 64 = softmax denom)
  yT_norm = yT[0:64] / broadcast(yT[64])

k-tiles are processed in pairs sharing one 2-bank PSUM tile so each EXP
covers 1024 columns (the ACT engine has a ~352-cycle fixed cost per
instruction). The AllGather is split into 4 chunks (per batch x half) so
gather and output projection overlap the remaining attention compute.

Inputs are bf16 (host-side cast); accumulation is f32 in PSUM.
"""

import numpy as np
import ml_dtypes

import concourse.bacc as bacc
import concourse.mybir as mybir
import concourse.tile as tile
from concourse.bass_utils import run_bass_kernel_spmd
from concourse.masks import make_identity

N_CORES = 8
B, T, C, H = 2, 2048, 1024, 16
D = 64                # head dim
HL = H // N_CORES     # heads per core = 2
DL = HL * D           # local feature dim = 128
TT = B * T            # 4096 tokens total
P = 128
NCH = C // P          # 8 contraction chunks
QCH = 512             # q-chunk (moving free dim)
NQC = T // QCH        # 4 q-chunks per batch entry
NKT = T // P          # 16 k-tiles per batch entry
HCH = T // 2          # AllGather chunk = half batch-entry = 1024 tokens
SCALE = 1.0 / np.sqrt(D)

BF = mybir.dt.bfloat16
F32 = mybir.dt.float32
AF = mybir.ActivationFunctionType


def build_graph():
    nc = bacc.Bacc("TRN2", target_bir_lowering=False, debug=False)

    xT = nc.dram_tensor("xT", [C, TT], BF, kind="ExternalInput")
    # all 4 weight shards pre-packed host-side into SBUF layout
    # [p, w, ci, m]: one contiguous 1MB DMA instead of 4x1024 tiny rows
    wall = nc.dram_tensor("wall", [P, 4 * NCH * DL], BF, kind="ExternalInput")
    out = nc.dram_tensor("out", [DL, TT], F32, kind="ExternalOutput")

    with tile.TileContext(nc) as tc:
        with (
            tc.tile_pool(name="sb", bufs=1) as sb,
            tc.tile_pool(name="ps", bufs=1, space="PSUM") as ps,
            tc.tile_pool(name="dram", bufs=1, space="DRAM") as dram,
        ):
            # ---- phase 0: loads ----
            w_sb = sb.tile([P, 4, NCH, DL], BF, name="w_sb")
            nc.sync.dma_start(
                w_sb[:], wall[:].rearrange("p (w a m) -> p w a m", w=4, a=NCH)
            )
            wq_sb, wk_sb, wv_sb, wo_sb = (w_sb[:, i] for i in range(4))

            ident = sb.tile([P, P], BF, name="ident")
            make_identity(nc, ident)
            # strictly-lower-triangular -1e9 (k > q) used to mask causal
            # logits on the diagonal blocks, injected into the St PSUM
            # accumulation group via matmul(ident, mneg)
            mneg = sb.tile([P, P], BF, name="mneg")
            nc.gpsimd.memset(mneg[:], 0.0)
            nc.gpsimd.affine_select(
                out=mneg[:],
                in_=mneg[:],
                compare_op=mybir.AluOpType.is_ge,
                fill=-1e9,
                base=0,
                channel_multiplier=-1,
                pattern=[[1, P]],
            )
            # [1, 64] f32 ones: K=1 outer-product broadcast of the softmax
            # denominator across partitions on the TensorEngine
            ones64 = sb.tile([1, D], BF, name="ones64")
            nc.vector.memset(ones64[:], 1.0)

            qT_sb = sb.tile([P, TT], BF, name="qT_sb")
            kT_sb = sb.tile([P, TT], BF, name="kT_sb")
            vT_sb = sb.tile([P, TT], BF, name="vT_sb")
            # v in natural layout, packed per 128-token tile as
            # [headA(64) | 1 | headB(64) | 1] -> 130 columns
            v_sb = sb.tile([P, TT // P, 2 * (D + 1)], BF, name="v_sb")
            nc.gpsimd.memset(v_sb[:], 1.0)

            yT_all = sb.tile([P, TT], BF, name="yT_all")

            with tc.tile_pool(name="xp", bufs=1) as xp:
                xT_sb = xp.tile([P, NCH, TT], BF, name="xT_sb")
                # tch-major, 512-col pieces: the first projection's operands
                # land after ~8us and later pieces stream behind compute
                for tch in range(TT // QCH):
                    tsl = slice(tch * QCH, (tch + 1) * QCH)
                    for ci in range(NCH):
                        nc.sync.dma_start(
                            xT_sb[:, ci, tsl], xT[ci * P:(ci + 1) * P, tsl]
                        )

                # ---- phase 1: QKV projections (transposed layout) ----
                for tch in range(TT // QCH):
                    tsl = slice(tch * QCH, (tch + 1) * QCH)
                    for wsb, dst in (
                        (wq_sb, qT_sb), (wk_sb, kT_sb), (wv_sb, vT_sb)
                    ):
                        pj = ps.tile([P, QCH], F32, tag="st", bufs=2)
                        for ci in range(NCH):
                            nc.tensor.matmul(
                                pj[:],
                                wsb[:, ci, :],
                                xT_sb[:, ci, tsl],
                                start=(ci == 0),
                                stop=(ci == NCH - 1),
                            )
                        nc.vector.tensor_copy(dst[:, tsl], pj[:])
                    # transpose this chunk of vT into v_sb (natural layout)
                    for t32 in range(tch * (QCH // P), (tch + 1) * (QCH // P)):
                        tr = ps.tile([P, P], BF, tag="st", bufs=2)
                        nc.tensor.transpose(
                            tr[:], vT_sb[:, t32 * P:(t32 + 1) * P], ident[:]
                        )
                        out_ap = v_sb[:, t32, :].rearrange(
                            "p (h x) -> p h x", h=HL
                        )[:, :, 0:D]
                        in_ap = tr[:].rearrange("p (h x) -> p h x", h=HL)
                        nc.vector.tensor_copy(out_ap, in_ap)

                # ---- phase 2+3+4: attention, chunked AllGather, O-proj ----
                CHUNKS = [(0, HCH), (HCH, HCH), (2 * HCH, HCH),
                          (3 * HCH, QCH), (3 * HCH + QCH, QCH)]
                ag_in = [
                    dram.tile([DL, cw], BF, name=f"ag_in{c}")
                    for c, (c0, cw) in enumerate(CHUNKS)
                ]
                ytf = [
                    dram.tile([C, cw], BF, name=f"ytf{c}", addr_space="Shared")
                    for c, (c0, cw) in enumerate(CHUNKS)
                ]

                def attn_compute(b, jq, h):
                    rsl = slice(h * D, (h + 1) * D)
                    q0 = b * T + jq * QCH
                    yt = ps.tile([D + 1, QCH], F32, tag="yt", bufs=2,
                                 name=f"yt_{b}_{jq}_{h}")
                    nkt = 4 * jq + 4
                    for pr in range(nkt // 2):
                        st = ps.tile([P, 2 * QCH], F32, tag="st", bufs=2)
                        pt = sb.tile([P, 2 * QCH], BF, tag="pt", bufs=3)
                        for half in range(2):
                            kt = 2 * pr + half
                            k0 = b * T + kt * P
                            i = kt - 4 * jq
                            # diagonal tiles: only q >= kt*128 is live; the
                            # leading 128 live columns get the -1e9 triangle
                            qv = max(i, 0) * P
                            ssl = slice(half * QCH + qv, (half + 1) * QCH)
                            nc.tensor.matmul(
                                st[:, ssl],
                                kT_sb[rsl, k0:k0 + P],
                                qT_sb[rsl, q0 + qv:q0 + QCH],
                                start=True,
                                stop=(i < 0),
                            )
                            if i >= 0:
                                nc.tensor.matmul(
                                    st[:, half * QCH + qv:
                                       half * QCH + qv + P],
                                    ident[:],
                                    mneg[:],
                                    start=False,
                                    stop=True,
                                )
                        nc.scalar.activation(
                            pt[:], st[:], AF.Exp, scale=float(SCALE)
                        )
                        for half in range(2):
                            kt = 2 * pr + half
                            qv = max(kt - 4 * jq, 0) * P
                            nc.tensor.matmul(
                                yt[:, qv:QCH],
                                v_sb[:, b * NKT + kt,
                                     h * (D + 1):(h + 1) * (D + 1)],
                                pt[:, half * QCH + qv:(half + 1) * QCH],
                                start=(kt == 0),
                                stop=(kt == nkt - 1),
                            )
                    # denominator row -> SBUF (bf16) immediately; the rest of
                    # the eviction is deferred so the PE queue never waits
                    den = sb.tile([1, QCH], BF, tag="den", bufs=4)
                    nc.vector.tensor_copy(den[:], yt[D:D + 1, :])
                    return yt, den

                def attn_evict(b, jq, h, yt, den):
                    rsl = slice(h * D, (h + 1) * D)
                    q0 = b * T + jq * QCH
                    # broadcast denominator across partitions via a K=1 bf16
                    # outer product on the TensorEngine
                    bc = ps.tile([D, QCH], F32, tag="st", bufs=2)
                    nc.tensor.matmul(
                        bc[:], ones64[:], den[:], start=True, stop=True
                    )
                    rcp = sb.tile([D, QCH], F32, tag="rcp", bufs=3)
                    scr = sb.tile([D, QCH], F32, tag="scr", bufs=3)
                    nc.vector.reciprocal_approx_accurate(
                        rcp[:], bc[:], scratch=scr[:]
                    )
                    nc.vector.tensor_mul(
                        yT_all[rsl, q0:q0 + QCH], yt[0:D, :], rcp[:]
                    )

                def ag_issue(c):
                    c0, cw = CHUNKS[c]
                    nc.sync.dma_start(ag_in[c][:], yT_all[:, c0:c0 + cw])
                    nc.gpsimd.collective_compute(
                        "AllGather",
                        mybir.AluOpType.bypass,
                        replica_groups=[list(range(N_CORES))],
                        ins=[ag_in[c][:]],
                        outs=[ytf[c][:]],
                    )

                def oproj_compute(c):
                    c0, cw = CHUNKS[c]
                    yf = sb.tile([P, NCH, HCH], BF, tag="yf", bufs=2)
                    for ci in range(NCH):
                        nc.sync.dma_start(
                            yf[:, ci, 0:cw], ytf[c][ci * P:(ci + 1) * P, :]
                        )
                    for tch in range(cw // QCH):
                        po = ps.tile([P, QCH], F32, tag="po", bufs=2)
                        for ci in range(NCH):
                            nc.tensor.matmul(
                                po[:],
                                wo_sb[:, ci, :],
                                yf[:, ci, tch * QCH:(tch + 1) * QCH],
                                start=(ci == 0),
                                stop=(ci == NCH - 1),
                            )
                        ob = sb.tile([P, QCH], F32, tag="ob", bufs=2)
                        nc.vector.tensor_copy(ob[:], po[:])
                        nc.sync.dma_start(
                            out[:, c0 + tch * QCH:c0 + (tch + 1) * QCH],
                            ob[:],
                        )

                # gather chunks (token offset, width); the tail is split
                # finer so the final gather+oproj exposure is small
                for b in range(B):
                    for jq in range(NQC):
                        pend = []
                        for h in range(HL):
                            pend.append((b, jq, h) + attn_compute(b, jq, h))
                        for b_, jq_, h_, yt_, den_ in pend:
                            attn_evict(b_, jq_, h_, yt_, den_)
                        step = (b, jq)
                        # issue gathers as soon as their tokens are done;
                        # compute each O-proj chunk one gather later so its
                        # PE work never head-of-line blocks on the collective
                        if step == (0, 1):
                            ag_issue(0)
                        elif step == (0, 3):
                            ag_issue(1)
                            oproj_compute(0)
                        elif step == (1, 1):
                            ag_issue(2)
                            oproj_compute(1)
                        elif step == (1, 2):
                            ag_issue(3)
                            oproj_compute(2)
                        elif step == (1, 3):
                            ag_issue(4)
                            oproj_compute(3)
                            oproj_compute(4)

    nc.finalize()
    return nc


_GRAPH = None


def _get_graph():
    global _GRAPH
    if _GRAPH is None:
        _GRAPH = build_graph()
    return _GRAPH


def prepare_in_maps(x, Wq, Wk, Wv, Wo):
    x = np.asarray(x, np.float32)
    Wq = np.asarray(Wq, np.float32)
    Wk = np.asarray(Wk, np.float32)
    Wv = np.asarray(Wv, np.float32)
    Wo = np.asarray(Wo, np.float32)

    bf = ml_dtypes.bfloat16
    xTh = np.ascontiguousarray(x.reshape(TT, C).T).astype(bf)
    in_maps = []
    for r in range(N_CORES):
        sl = slice(r * DL, (r + 1) * DL)
        # pack the 4 transposed weight shards into the SBUF layout
        # [p, w, ci, m] where the shard row index is c = ci*128 + p
        wall = np.empty((P, 4, NCH, DL), np.float32)
        for w, W in enumerate((Wq, Wk, Wv, Wo)):
            wall[:, w] = W[sl].T.reshape(NCH, P, DL).transpose(1, 0, 2)
        in_maps.append({
            "xT": xTh,
            "wall": np.ascontiguousarray(
                wall.reshape(P, 4 * NCH * DL)).astype(bf),
        })
    return in_maps


def assemble_output(results):
    outT = np.concatenate(
        [np.asarray(results[r]["out"], np.float32) for r in range(N_CORES)],
        axis=0,
    )  # [C, TT]
    return np.ascontiguousarray(outT.T).reshape(B, T, C)


def kernel(x, Wq, Wk, Wv, Wo):
    nc = _get_graph()
    in_maps = prepare_in_maps(x, Wq, Wk, Wv, Wo)
    res = run_bass_kernel_spmd(nc, in_maps, core_ids=list(range(N_CORES)))
    return assemble_output(res.results)


# revision 15
# speedup vs baseline: 1.3771x; 1.1319x over previous
"""Causal multi-head attention block (B=2, T=2048, C=1024, H=16) on 8 TRN2
NeuronCores.

Sharding: Megatron-style tensor parallel over heads. Core r owns heads
{2r, 2r+1} (output dims [128r, 128r+128) of Wq/Wk/Wv). The final output
projection contracts over all of C, so cores AllGather their local yT
shards (concat on the partition axis == feature axis) into yT_full
[C, B*T], then each core computes a 128-column shard of the output:
outT_shard = Wo[128r:128r+128, :] @ yT_full.

Everything on-device is computed in the "transposed" orientation
(feature-major, token-minor) so the TensorEngine contraction axis always
sits on SBUF partitions and the softmax denominator arrives for free via
a ones-column appended to V:

  qT/kT/vT [128, 4096] = W_shard @ x^T          (x^T passed from host)
  ST tile [128k, 512q] = kT_slice.T @ qT_slice  (contract d=64)
  PT = exp(ST * 1/sqrt(d))                      (no max-subtraction: logits
                                                 are ~N(0,1), |S|max ~ 6)
  causal mask: zero PT where k > q via gpsimd.affine_select
  yT [65, 512] += [v | 1].T @ PT                (row 64 = softmax denom)
  yT_norm = yT[0:64] / broadcast(yT[64])

k-tiles are processed in pairs sharing one 2-bank PSUM tile so each EXP
covers 1024 columns (the ACT engine has a ~352-cycle fixed cost per
instruction). The AllGather is split into 4 chunks (per batch x half) so
gather and output projection overlap the remaining attention compute.

Inputs are bf16 (host-side cast); accumulation is f32 in PSUM.
"""

import numpy as np
import ml_dtypes

import concourse.bacc as bacc
import concourse.mybir as mybir
import concourse.tile as tile
from concourse.bass_utils import run_bass_kernel_spmd
from concourse.masks import make_identity

N_CORES = 8
B, T, C, H = 2, 2048, 1024, 16
D = 64                # head dim
HL = H // N_CORES     # heads per core = 2
DL = HL * D           # local feature dim = 128
TT = B * T            # 4096 tokens total
P = 128
NCH = C // P          # 8 contraction chunks
QCH = 512             # q-chunk (moving free dim)
NQC = T // QCH        # 4 q-chunks per batch entry
NKT = T // P          # 16 k-tiles per batch entry
HCH = T // 2          # AllGather chunk = half batch-entry = 1024 tokens
SCALE = 1.0 / np.sqrt(D)

BF = mybir.dt.bfloat16
F32 = mybir.dt.float32
AF = mybir.ActivationFunctionType


def build_graph():
    nc = bacc.Bacc("TRN2", target_bir_lowering=False, debug=False)

    xT = nc.dram_tensor("xT", [C, TT], BF, kind="ExternalInput")
    # all 4 weight shards pre-packed host-side into SBUF layout
    # [p, w, ci, m]: contiguous rows, loaded as 8 parallel DMAs
    wall = nc.dram_tensor("wall", [P, 4 * NCH * DL], BF, kind="ExternalInput")
    out = nc.dram_tensor("out", [DL, TT], F32, kind="ExternalOutput")

    with tile.TileContext(nc) as tc:
        with (
            tc.tile_pool(name="sb", bufs=1) as sb,
            tc.tile_pool(name="ps", bufs=1, space="PSUM") as ps,
            tc.tile_pool(name="dram", bufs=1, space="DRAM") as dram,
        ):
            # ---- loads ----
            w_sb = sb.tile([P, 4 * NCH * DL], BF, name="w_sb")
            WCOLS = 4 * NCH * DL
            for pc in range(8):
                csl = slice(pc * (WCOLS // 8), (pc + 1) * (WCOLS // 8))
                nc.sync.dma_start(w_sb[:, csl], wall[:, csl])
            w4 = w_sb[:].rearrange("p (w a m) -> p w a m", w=4, a=NCH)
            wq_sb, wk_sb, wv_sb, wo_sb = (w4[:, i] for i in range(4))

            ident = sb.tile([P, P], BF, name="ident")
            make_identity(nc, ident)
            # strictly-lower-triangular -1e9 (k > q): masks causal logits on
            # diagonal blocks, injected into the St PSUM group via
            # matmul(ident, mneg)
            mneg = sb.tile([P, P], BF, name="mneg")
            nc.gpsimd.memset(mneg[:], 0.0)
            nc.gpsimd.affine_select(
                out=mneg[:], in_=mneg[:],
                compare_op=mybir.AluOpType.is_ge,
                fill=-1e9, base=0, channel_multiplier=-1, pattern=[[1, P]],
            )
            # [1, 64] bf16 ones: K=1 outer-product broadcast of the softmax
            # denominator across partitions on the TensorEngine
            ones64 = sb.tile([1, D], BF, name="ones64")
            nc.vector.memset(ones64[:], 1.0)

            qT_sb = sb.tile([P, TT], BF, name="qT_sb")
            kT_sb = sb.tile([P, TT], BF, name="kT_sb")
            vT_sb = sb.tile([P, TT], BF, name="vT_sb")
            # v in natural layout, packed per 128-token tile as
            # [headA(64) | 1 | headB(64) | 1] -> 130 columns
            v_sb = sb.tile([P, TT // P, 2 * (D + 1)], BF, name="v_sb")
            nc.gpsimd.memset(v_sb[:], 1.0)

            # gather chunks (token offset, width); tail split finer
            CHUNKS = [(0, HCH), (HCH, HCH), (2 * HCH, HCH),
                      (3 * HCH, QCH), (3 * HCH + QCH, QCH)]
            ag_in = [
                dram.tile([DL, cw], BF, name=f"ag_in{c}")
                for c, (c0, cw) in enumerate(CHUNKS)
            ]
            ytf = [
                dram.tile([C, cw], BF, name=f"ytf{c}", addr_space="Shared")
                for c, (c0, cw) in enumerate(CHUNKS)
            ]
            # (b, jq) -> (chunk, col offset within chunk)
            CHUNK_OF = {(0, 0): (0, 0), (0, 1): (0, QCH),
                        (0, 2): (1, 0), (0, 3): (1, QCH),
                        (1, 0): (2, 0), (1, 1): (2, QCH),
                        (1, 2): (3, 0), (1, 3): (4, 0)}

            with tc.tile_pool(name="xp", bufs=1) as xp:
                xT_sb = xp.tile([P, NCH, TT], BF, name="xT_sb")
                # tch-major, 512-col pieces: first projection operands land
                # after ~8us, later pieces stream behind compute
                for tch in range(TT // QCH):
                    tsl = slice(tch * QCH, (tch + 1) * QCH)
                    for ci in range(NCH):
                        nc.sync.dma_start(
                            xT_sb[:, ci, tsl], xT[ci * P:(ci + 1) * P, tsl]
                        )

                def proj_group(tch, wsb, dst):
                    tsl = slice(tch * QCH, (tch + 1) * QCH)
                    pj = ps.tile([P, QCH], F32, tag="pjpo", bufs=2,
                                 name="pj")
                    for ci in range(NCH):
                        nc.tensor.matmul(
                            pj[:], wsb[:, ci, :], xT_sb[:, ci, tsl],
                            start=(ci == 0), stop=(ci == NCH - 1),
                        )
                    nc.vector.tensor_copy(dst[:, tsl], pj[:])

                def vtrans(t32):
                    tr = ps.tile([P, P], BF, tag="pjpo", bufs=2, name="tr")
                    nc.tensor.transpose(
                        tr[:], vT_sb[:, t32 * P:(t32 + 1) * P], ident[:]
                    )
                    out_ap = v_sb[:, t32, :].rearrange(
                        "p (h x) -> p h x", h=HL
                    )[:, :, 0:D]
                    in_ap = tr[:].rearrange("p (h x) -> p h x", h=HL)
                    nc.vector.tensor_copy(out_ap, in_ap)

                def attn_compute(b, jq, h):
                    rsl = slice(h * D, (h + 1) * D)
                    q0 = b * T + jq * QCH
                    yt = ps.tile([D + 1, QCH], F32, tag="yt", bufs=2,
                                 name="yt")
                    nkt = 4 * jq + 4
                    for pr in range(nkt // 2):
                        st = ps.tile([P, 2 * QCH], F32, tag="st", bufs=2,
                                     name="st")
                        pt = sb.tile([P, 2 * QCH], BF, tag="pt", bufs=3,
                                     name="pt")
                        for half in range(2):
                            kt = 2 * pr + half
                            k0 = b * T + kt * P
                            i = kt - 4 * jq
                            # diagonal tiles: only q >= kt*128 live; leading
                            # 128 live columns get the -1e9 triangle
                            qv = max(i, 0) * P
                            ssl = slice(half * QCH + qv, (half + 1) * QCH)
                            nc.tensor.matmul(
                                st[:, ssl],
                                kT_sb[rsl, k0:k0 + P],
                                qT_sb[rsl, q0 + qv:q0 + QCH],
                                start=True, stop=(i < 0),
                            )
                            if i >= 0:
                                nc.tensor.matmul(
                                    st[:, half * QCH + qv:
                                       half * QCH + qv + P],
                                    ident[:], mneg[:],
                                    start=False, stop=True,
                                )
                        nc.scalar.activation(
                            pt[:], st[:], AF.Exp, scale=float(SCALE)
                        )
                        for half in range(2):
                            kt = 2 * pr + half
                            qv = max(kt - 4 * jq, 0) * P
                            nc.tensor.matmul(
                                yt[:, qv:QCH],
                                v_sb[:, b * NKT + kt,
                                     h * (D + 1):(h + 1) * (D + 1)],
                                pt[:, half * QCH + qv:(half + 1) * QCH],
                                start=(kt == 0), stop=(kt == nkt - 1),
                            )
                    # denominator row -> SBUF bf16 right away; the rest of
                    # the eviction runs after the next filler block so the
                    # PE queue never waits on it
                    den = sb.tile([1, QCH], BF, tag="den", bufs=4, name="den")
                    nc.vector.tensor_copy(den[:], yt[D:D + 1, :])
                    return yt, den

                def attn_evict(b, jq, h, yt, den):
                    rsl = slice(h * D, (h + 1) * D)
                    bc = ps.tile([D, QCH], F32, tag="pjpo", bufs=2, name="bc")
                    nc.tensor.matmul(
                        bc[:], ones64[:], den[:], start=True, stop=True
                    )
                    rcp = sb.tile([D, QCH], F32, tag="rcp", bufs=3, name="rcp")
                    scr = sb.tile([D, QCH], F32, tag="scr", bufs=3, name="scr")
                    nc.vector.reciprocal_approx_accurate(
                        rcp[:], bc[:], scratch=scr[:]
                    )
                    yn = sb.tile([D, QCH], BF, tag="yn", bufs=4, name="yn")
                    nc.vector.tensor_mul(yn[:], yt[0:D, :], rcp[:])
                    # stream this piece straight into the gather input
                    c, off = CHUNK_OF[(b, jq)]
                    for s in range(2):
                        nc.sync.dma_start(
                            ag_in[c][h * D:(h + 1) * D,
                                     off + s * (QCH // 2):
                                     off + (s + 1) * (QCH // 2)],
                            yn[:, s * (QCH // 2):(s + 1) * (QCH // 2)],
                        )

                def ag_fire(c):
                    nc.gpsimd.collective_compute(
                        "AllGather",
                        mybir.AluOpType.bypass,
                        replica_groups=[list(range(N_CORES))],
                        ins=[ag_in[c][:]],
                        outs=[ytf[c][:]],
                    )

                yf_tiles = {}

                def yf_load(c):
                    c0, cw = CHUNKS[c]
                    yf = sb.tile([P, NCH, HCH], BF, tag="yf", bufs=2,
                                 name="yf")
                    yf_tiles[c] = yf
                    for ci in range(NCH):
                        for s in range(2):
                            nc.sync.dma_start(
                                yf[:, ci, s * (cw // 2):(s + 1) * (cw // 2)],
                                ytf[c][ci * P:(ci + 1) * P,
                                       s * (cw // 2):(s + 1) * (cw // 2)],
                            )

                def po_group(c, tch):
                    c0, cw = CHUNKS[c]
                    yf = yf_tiles[c]
                    po = ps.tile([P, QCH], F32, tag="pjpo", bufs=2, name="po")
                    for ci in range(NCH):
                        nc.tensor.matmul(
                            po[:], wo_sb[:, ci, :],
                            yf[:, ci, tch * QCH:(tch + 1) * QCH],
                            start=(ci == 0), stop=(ci == NCH - 1),
                        )
                    ob = sb.tile([P, QCH], F32, tag="ob", bufs=2, name="ob")
                    nc.vector.tensor_copy(ob[:], po[:])
                    for s in range(4):
                        o0 = c0 + tch * QCH + s * (QCH // 4)
                        nc.sync.dma_start(
                            out[:, o0:o0 + QCH // 4],
                            ob[:, s * (QCH // 4):(s + 1) * (QCH // 4)],
                        )

                # ---- prologue: b0 projections ----
                for tch in range(4):
                    for wsb, dst in ((wq_sb, qT_sb), (wk_sb, kT_sb),
                                     (wv_sb, vT_sb)):
                        proj_group(tch, wsb, dst)
                    for t32 in range(tch * 4, tch * 4 + 4):
                        vtrans(t32)

                # filler: b1 projections, fed into b0's attention stream to
                # keep the PE dense (HAM warm) while exp gates the AV chain
                filler = []
                for tch in range(4, 8):
                    for wsb, dst in ((wq_sb, qT_sb), (wk_sb, kT_sb),
                                     (wv_sb, vT_sb)):
                        filler.append((proj_group, (tch, wsb, dst)))
                    for t32 in range(tch * 4, tch * 4 + 4):
                        filler.append((vtrans, (t32,)))

                def pop_filler(n):
                    for _ in range(min(n, len(filler))):
                        fn, args = filler.pop(0)
                        fn(*args)

                # ---- b0 attention ----
                for jq in range(NQC):
                    for h in range(HL):
                        yt, den = attn_compute(0, jq, h)
                        pop_filler(jq + 1)
                        attn_evict(0, jq, h, yt, den)
                    if jq == 1:
                        ag_fire(0)
                    elif jq == 3:
                        pop_filler(99)
                        ag_fire(1)

                # ---- b1 attention with O-proj filler ----
                for jq in range(NQC):
                    for h in range(HL):
                        yt, den = attn_compute(1, jq, h)
                        # staggered O-proj: each chunk's PE work lands well
                        # after its collective completed
                        step = (jq, h)
                        if step == (0, 0):
                            yf_load(0); po_group(0, 0)
                        elif step == (0, 1):
                            po_group(0, 1)
                        elif step == (1, 1):
                            yf_load(1); po_group(1, 0)
                        elif step == (2, 0):
                            po_group(1, 1)
                        elif step == (2, 1):
                            yf_load(2); po_group(2, 0)
                        elif step == (3, 0):
                            po_group(2, 1)
                        elif step == (3, 1):
                            yf_load(3); po_group(3, 0)
                        attn_evict(1, jq, h, yt, den)
                    if jq == 1:
                        ag_fire(2)
                    elif jq == 2:
                        ag_fire(3)
                    elif jq == 3:
                        ag_fire(4)
                        yf_load(4)
                        po_group(4, 0)

    nc.finalize()
    return nc


_GRAPH = None


def _get_graph():
    global _GRAPH
    if _GRAPH is None:
        _GRAPH = build_graph()
    return _GRAPH


def prepare_in_maps(x, Wq, Wk, Wv, Wo):
    x = np.asarray(x, np.float32)
    Wq = np.asarray(Wq, np.float32)
    Wk = np.asarray(Wk, np.float32)
    Wv = np.asarray(Wv, np.float32)
    Wo = np.asarray(Wo, np.float32)

    bf = ml_dtypes.bfloat16
    xTh = np.ascontiguousarray(x.reshape(TT, C).T).astype(bf)
    in_maps = []
    for r in range(N_CORES):
        sl = slice(r * DL, (r + 1) * DL)
        # pack the 4 transposed weight shards into the SBUF layout
        # [p, w, ci, m] where the shard row index is c = ci*128 + p
        wall = np.empty((P, 4, NCH, DL), np.float32)
        for w, W in enumerate((Wq, Wk, Wv, Wo)):
            wall[:, w] = W[sl].T.reshape(NCH, P, DL).transpose(1, 0, 2)
        in_maps.append({
            "xT": xTh,
            "wall": np.ascontiguousarray(
                wall.reshape(P, 4 * NCH * DL)).astype(bf),
        })
    return in_maps


def assemble_output(results):
    outT = np.concatenate(
        [np.asarray(results[r]["out"], np.float32) for r in range(N_CORES)],
        axis=0,
    )  # [C, TT]
    return np.ascontiguousarray(outT.T).reshape(B, T, C)


def kernel(x, Wq, Wk, Wv, Wo):
    nc = _get_graph()
    in_maps = prepare_in_maps(x, Wq, Wk, Wv, Wo)
    res = run_bass_kernel_spmd(nc, in_maps, core_ids=list(range(N_CORES)))
    return assemble_output(res.results)
